# revision 1
# baseline (speedup 1.0000x reference)
"""BitNet linear (y = (x @ sign(W).T + b) * mean(|W|)) on 8 trn2 NeuronCores.

Sharding: column-parallel — W is sharded along out_features across the 8
cores, x is replicated, each core produces out[:, shard] and the host
concatenates.

Device algorithm (per core):
  1. DMA the core's W^T shard (f32), compute w_q = sign(W) in bf16 via a
     mult/min/max clamp (exact {-1,0,+1}), and per-chunk |W| sums.
  2. Partition-reduce the |W| sums with a ones-matmul (broadcasts the sum to
     all 128 partitions), AllReduce across the 8 cores, scale -> alpha.
  3. Main matmul: x is split into bf16 hi+lo (x = hi + lo + O(2^-16 x)), so
     two bf16 matmul passes accumulate an ~fp32-accurate product in PSUM at
     2 cycles/row instead of fp32 matmul's 4.  bias*alpha is broadcast once
     into a bf16 tile (ones-matmul) and fused into the PSUM->SBUF scale op
     (scalar_tensor_tensor: out = psum*alpha + bias*alpha); the next block's
     x DMA+split is emitted ahead of the copies so the in-order DVE stream
     keeps the PE fed across block boundaries.
"""

import numpy as np

import concourse.bass as bass
import concourse.mybir as mybir
import concourse.tile as tile
from concourse.bass import ds
from concourse.vector_clock import ScopedClock

# ---------------------------------------------------------------------------
# Compatibility patch: the pinned walrus (neuronxcc) in this container only
# supports ONE ge-wait per instruction and no eq-waits; the concourse Tile
# tail emits a Drain with multiple waits plus an eq-wait barrier butterfly
# ("Too many sync wait commands").  Replace the tail with one-wait-per-nop
# splitting and the NRT-expanded PSEUDO_SYNC_BARRIER (the pre-butterfly
# mechanism this walrus/NRT pair supports).
# ---------------------------------------------------------------------------


def _compat_drain_and_barrier(self, tick_clock, wait_clock):
    nc = self.nc
    coll = nc.sync.nop(nofuse=True)
    wait_clock.add_sem_waits(coll.ins, ScopedClock({None: tick_clock.global_clock}))
    si = coll.ins.sync_info
    if si is not None:
        waits = list(si.on_wait)
        if len(waits) > 1:
            coll.ins.sync_info = mybir.SyncInfo(
                on_wait=[waits[0]], on_update=list(si.on_update)
            )
            for w in waits[1:]:
                extra = nc.sync.nop(nofuse=True)
                extra.ins.sync_info = mybir.SyncInfo(on_wait=[w], on_update=[])
    for eng in nc.engines.values():
        eng.drain()
    nc._nrt_pseudo_barrier()
    popped = nc._tile_sem_poison_stack.pop()
    assert popped is self._sem_poison
    nc.clear_and_free_semaphores(list(self.sems.allocated().values()))
    nc._nrt_pseudo_barrier()


tile.TileContext._drain_and_barrier = _compat_drain_and_barrier

_legalize_ctr = [0]


def legalize_waits(nc):
    """Split instructions carrying more than the HW-supported number of sem
    waits (1; EventSemaphore: 2) into preceding one-wait NoOps on the same
    engine — semantically identical, encodable by the pinned walrus."""
    import bass_rust

    for f in nc.m.functions:
        for bb in f.blocks:
            il = bb.instructions
            i = 0
            while i < len(il):
                ins = il[i]
                si = ins.sync_info
                waits = list(si.on_wait) if si is not None else []
                limit = 2 if type(ins).__name__ == "InstEventSemaphore" else 1
                if len(waits) > limit:
                    keep = waits[-limit:]
                    spill = waits[:-limit]
                    for w in spill:
                        _legalize_ctr[0] += 1
                        nop = bass_rust.InstNoOp(
                            name=f"I-lw{_legalize_ctr[0]}", ins=[], outs=[]
                        )
                        nop.engine = ins.engine
                        nop.sync_info = mybir.SyncInfo(on_wait=[w], on_update=[])
                        il.insert(i, nop)
                        i += 1
                    ins.sync_info = mybir.SyncInfo(
                        on_wait=keep, on_update=list(si.on_update)
                    )
                i += 1


def elide_redundant_ldweights(nc):
    """Drop InstLdweights that reload the exact weights already sitting in
    the PE array.  bass lowers every InstMatmult to an Ldweights+Matmult
    pair; consecutive matmuls sharing one stationary tile reload it each
    time (~107ns of PE time apiece).  Two Ldweights with no other Ldweights
    between them and the same (tile name, offset, pattern) provably load
    identical content — tile names are unique per pool.tile() call and each
    tile is written before its first consumer only.  Elided instructions
    carrying semaphore waits/updates become NoOps to preserve sync."""
    import bass_rust

    n_elided = 0
    for f in nc.m.functions:
        for bb in f.blocks:
            il = bb.instructions
            last_key = None
            for i in range(len(il)):
                ins = il[i]
                nm = type(ins).__name__
                if nm != "InstLdweights":
                    continue
                a = ins.ins[0]
                bap = getattr(a, "bass_ap", None)
                if bap is None:
                    last_key = None
                    continue
                key = (
                    bap.tensor.name,
                    bap.offset,
                    str(bap.ap),
                    ins.perf_mode,
                    ins.is_transpose,
                    ins.tile_position,
                )
                if key == last_key:
                    si = ins.sync_info
                    has_sync = si is not None and (
                        list(si.on_wait) or list(si.on_update)
                    )
                    nop = bass_rust.InstNoOp(name=f"{ins.name}-eld", ins=[], outs=[])
                    nop.engine = ins.engine
                    if has_sync:
                        nop.sync_info = mybir.SyncInfo(
                            on_wait=list(si.on_wait), on_update=list(si.on_update)
                        )
                    il[i] = nop
                    n_elided += 1
                else:
                    last_key = key
    return n_elided


F32 = mybir.dt.float32
BF16 = mybir.dt.bfloat16

P = 128  # partitions


def build_bitnet_nc(
    M: int,
    K: int,
    N_shard: int,
    n_total_weight: int,
    n_cores: int = 8,
    nsplits: int = 2,
    debug: bool = False,
    legalize: bool = True,
    reps: int = 1,
    skip_cc: bool = False,
    pipeline_splits: bool = True,
    fuse_bias: bool = True,
):
    """Build the per-core Bass program.

    M: rows of x (B*S), K: in_features, N_shard: out_features per core.
    n_total_weight: total element count of the full W (for mean(|W|)).
    """
    assert M % P == 0 and K % P == 0
    K_CHUNKS = K // P
    KO = min(4, K_CHUNKS)  # k-chunks fetched per x DMA
    assert K_CHUNKS % KO == 0
    KK = K_CHUNKS // KO
    N_TILE = min(512, N_shard)
    assert N_shard % N_TILE == 0
    NB = N_shard // N_TILE
    M_BLOCKS = M // P

    nc = bass.Bass(num_devices=n_cores)
    xT = nc.declare_dram_parameter("xT", [K, M], F32, isOutput=False)
    wT = nc.declare_dram_parameter("wT", [K, N_shard], F32, isOutput=False)
    bias_d = nc.declare_dram_parameter("bias", [N_shard], F32, isOutput=False)
    out_d = nc.declare_dram_parameter("out", [M, N_shard], F32, isOutput=True)
    if debug:
        alpha_dbg = nc.declare_dram_parameter("alpha_dbg", [P, 1], F32, isOutput=True)
        psum_dbg = nc.declare_dram_parameter(
            "psum_dbg", [P, N_shard], F32, isOutput=True
        )
        wq_dbg = nc.declare_dram_parameter(
            "wq_dbg", [P, K // P, N_shard], F32, isOutput=True
        )

    with tile.TileContext(nc) as tc:
        wq_pool = tc.tile_pool(name="wq", bufs=1)
        wstage = tc.tile_pool(name="wstage", bufs=2)
        small = tc.tile_pool(name="small", bufs=1)
        xstage = tc.tile_pool(name="xstage", bufs=2)
        xhi_pool = tc.tile_pool(name="xhi", bufs=2 * KK - 1)
        xlo_pool = tc.tile_pool(name="xlo", bufs=2 * KK - 1)
        out_pool = tc.tile_pool(name="outp", bufs=2)
        psum_pool = tc.tile_pool(name="psum", bufs=2, space="PSUM")
        apsum_pool = tc.tile_pool(name="apsum", bufs=1, space="PSUM")
        dram = tc.tile_pool(name="dram", bufs=1, space="DRAM")

        with (
            wq_pool as wq_p,
            wstage as wst_p,
            small as small_p,
            xstage as xst_p,
            xhi_pool as xhi_p,
            xlo_pool as xlo_p,
            out_pool as out_p,
            psum_pool as ps_p,
            apsum_pool as aps_p,
            dram as dram_p,
        ):
            # ---------------- Phase A: sign(W) + |W| partial sums ----------
            wq = wq_p.tile([P, K_CHUNKS, N_shard], BF16)  # resident w_q^T
            acc = small_p.tile([P, K_CHUNKS], F32)
            for k in range(K_CHUNKS):
                wst = wst_p.tile([P, N_shard], F32, tag="wst")
                nc.sync.dma_start(wst[:], wT[k * P : (k + 1) * P, :])
                # per-chunk |W| sum on ScalarE: activation(Abs) accumulates
                # the row sum into acc while DVE does the sign clamp
                abs_dump = out_p.tile([P, N_shard], F32, tag="osb", name="abs_dump")
                nc.scalar.activation(
                    abs_dump[:],
                    wst[:],
                    mybir.ActivationFunctionType.Abs,
                    accum_out=acc[:, k : k + 1],
                )
                # sign via clamp: s = max(min(w * 1e30, 1), -1), exact
                # {-1, 0, +1} (|w| > 1e-30 or w == 0 for any normal float);
                # second op runs in place on the wq slice
                nc.vector.tensor_scalar(
                    wq[:, k, :],
                    wst[:],
                    1e30,
                    1.0,
                    mybir.AluOpType.mult,
                    mybir.AluOpType.min,
                )
                nc.vector.tensor_scalar(
                    wq[:, k, :], wq[:, k, :], -1.0, None, mybir.AluOpType.max
                )

            # ---------------- Phase B: alpha = mean|W| over all cores ------
            asum = small_p.tile([P, 1], F32)
            nc.vector.reduce_sum(asum[:], acc[:], axis=mybir.AxisListType.X)
            ones_pp = small_p.tile([P, P], F32)
            nc.vector.memset(ones_pp[:], 1.0)
            aps = aps_p.tile([P, 1], F32)
            # ones^T @ asum: sum over partitions, broadcast to all partitions
            nc.tensor.matmul(aps[:], ones_pp[:], asum[:], start=True, stop=True)
            part_sum = small_p.tile([P, 1], F32)
            nc.vector.tensor_copy(part_sum[:], aps[:])

            cc_in = dram_p.tile([P, 1], F32)
            cc_out = dram_p.tile(
                [P, 1], F32, addr_space="Shared" if n_cores > 4 else "Local"
            )
            nc.sync.dma_start(cc_in[:], part_sum[:])
            if skip_cc:
                nc.sync.dma_start(cc_out[:], cc_in[:])
            else:
                nc.gpsimd.collective_compute(
                    "AllReduce",
                    mybir.AluOpType.add,
                    replica_groups=[list(range(n_cores))],
                    ins=[cc_in.opt()],
                    outs=[cc_out.opt()],
                )
            gsum = small_p.tile([P, 1], F32)
            nc.sync.dma_start(gsum[:], cc_out[:])
            alpha = small_p.tile([P, 1], F32)
            nc.vector.tensor_scalar_mul(alpha[:], gsum[:], 1.0 / float(n_total_weight))

            # bias: build a [128, N_shard] bf16 broadcast of bias*alpha ONCE
            # (ones-matmul broadcast) and fuse the add into the per-block
            # PSUM->SBUF scale op — generic in bias, no per-block PE cost
            bias_sb = small_p.tile([1, N_shard], F32)
            nc.sync.dma_start(bias_sb[:], bias_d[None, :])
            ones_row = small_p.tile([1, P], F32)
            nc.vector.memset(ones_row[:], 1.0)
            if fuse_bias:
                # scale bias by alpha in place (raw bias not needed again)
                nc.vector.tensor_scalar_mul(bias_sb[:], bias_sb[:], alpha[:1, :])
                bias_bc = small_p.tile([P, N_shard], BF16)
                for n in range(NB):
                    bps = ps_p.tile([P, N_TILE], F32, tag="ps", name=f"bps{n}")
                    nc.tensor.matmul(
                        bps[:],
                        ones_row[:],
                        bias_sb[:, ds(n * N_TILE, N_TILE)],
                        start=True,
                        stop=True,
                    )
                    nc.vector.tensor_copy(bias_bc[:, ds(n * N_TILE, N_TILE)], bps[:])

            # ---------------- Phase C: main matmul -------------------------
            # Software-pipelined: block b+1's x DMA + hi/lo split is emitted
            # BEFORE block b's PSUM->SBUF copies, so the in-order DVE stream
            # produces the next block's stationaries while the PE runs block
            # b's matmuls (otherwise DVE blocks on the copy's PSUM wait and
            # the PE idles ~3us per block boundary — enough to re-throttle
            # the HAM clock gate).
            xT_r = xT.rearrange("(kk ko p) m -> kk p ko m", p=P, ko=KO)
            total_blocks = reps * M_BLOCKS

            def emit_split(m, tag):
                his, los = [], []
                for kk in range(KK):
                    xs = xst_p.tile([P, KO, P], F32, tag="xs", name=f"xs{tag}_{kk}")
                    nc.sync.dma_start(xs[:], xT_r[kk, :, :, m * P : (m + 1) * P])
                    hi = xhi_p.tile(
                        [P, KO, P], BF16, tag="xhi", name=f"hi{tag}_{kk}"
                    )
                    nc.vector.tensor_copy(hi[:], xs[:])
                    his.append(hi)
                    if nsplits == 2:
                        lo = xlo_p.tile(
                            [P, KO, P], BF16, tag="xlo", name=f"lo{tag}_{kk}"
                        )
                        nc.vector.tensor_sub(lo[:], xs[:], hi[:])
                        los.append(lo)
                return his, los

            pending = emit_split(0, "b0") if pipeline_splits else None
            for bi in range(total_blocks):
                m = bi % M_BLOCKS
                if pipeline_splits:
                    his, los = pending
                else:
                    his, los = emit_split(m, f"b{bi}")

                psums = [
                    ps_p.tile([P, N_TILE], F32, tag="ps", name=f"ps{n}")
                    for n in range(NB)
                ]
                first = True
                for kk in range(KK):
                    for ko in range(KO):
                        k = kk * KO + ko
                        parts = [his[kk][:, ko, :]]
                        if nsplits == 2:
                            parts.append(los[kk][:, ko, :])
                        last = kk == KK - 1 and ko == KO - 1
                        for pi, lhsT in enumerate(parts):
                            stop_here = fuse_bias and last and pi == len(parts) - 1
                            for n in range(NB):
                                nc.tensor.matmul(
                                    psums[n][:],
                                    lhsT,
                                    wq[:, k, ds(n * N_TILE, N_TILE)],
                                    start=first,
                                    stop=stop_here,
                                )
                            first = False
                if not fuse_bias:
                    # bias via K=1 ones-matmul (also closes the accumulation)
                    for n in range(NB):
                        nc.tensor.matmul(
                            psums[n][:],
                            ones_row[:],
                            bias_sb[:, ds(n * N_TILE, N_TILE)],
                            start=False,
                            stop=True,
                        )

                if pipeline_splits and bi + 1 < total_blocks:
                    pending = emit_split((bi + 1) % M_BLOCKS, f"b{bi + 1}")

                osb = out_p.tile([P, N_shard], F32, tag="osb")
                if debug and m == 0 and bi == m:
                    psd = out_p.tile([P, N_shard], F32, tag="psd")
                    for n in range(NB):
                        nc.vector.tensor_copy(
                            psd[:, ds(n * N_TILE, N_TILE)], psums[n][:]
                        )
                    nc.sync.dma_start(psum_dbg[:, :], psd[:])
                for n in range(NB):
                    if fuse_bias:
                        nc.vector.scalar_tensor_tensor(
                            osb[:, ds(n * N_TILE, N_TILE)],
                            psums[n][:],
                            alpha[:],
                            bias_bc[:, ds(n * N_TILE, N_TILE)],
                            mybir.AluOpType.mult,
                            mybir.AluOpType.add,
                        )
                    else:
                        nc.vector.tensor_scalar_mul(
                            osb[:, ds(n * N_TILE, N_TILE)], psums[n][:], alpha[:]
                        )
                nc.sync.dma_start(out_d[m * P : (m + 1) * P, :], osb[:])

            if debug:
                nc.sync.dma_start(alpha_dbg[:, :], alpha[:])
                wqf = out_p.tile([P, K_CHUNKS, N_shard], F32, tag="wqf")
                nc.vector.tensor_copy(wqf[:], wq[:])
                nc.sync.dma_start(wq_dbg[:, :, :], wqf[:])

    if legalize:
        legalize_waits(nc)  # required for walrus; CoreSim chokes on raw NoOps
    return nc


def run_bitnet(
    x: np.ndarray,
    weight: np.ndarray,
    bias: np.ndarray,
    n_cores: int = 8,
    nsplits: int = 2,
    trace: bool = False,
):
    """Host driver: shard, run on n_cores, gather. x: [..., K], weight: [N, K]."""
    from concourse.bass_utils import run_bass_kernel_spmd

    lead_shape = x.shape[:-1]
    K = x.shape[-1]
    N = weight.shape[0]
    M = int(np.prod(lead_shape))
    assert weight.shape == (N, K) and bias.shape == (N,)
    assert N % n_cores == 0
    N_shard = N // n_cores

    x2 = np.ascontiguousarray(x.reshape(M, K).astype(np.float32, copy=False))
    xT = np.ascontiguousarray(x2.T)
    w = weight.astype(np.float32, copy=False)

    nc = build_bitnet_nc(M, K, N_shard, N * K, n_cores=n_cores, nsplits=nsplits)

    in_maps = []
    for c in range(n_cores):
        wTc = np.ascontiguousarray(w[c * N_shard : (c + 1) * N_shard, :].T)
        bc = np.ascontiguousarray(bias[c * N_shard : (c + 1) * N_shard]).astype(
            np.float32, copy=False
        )
        in_maps.append({"xT": xT, "wT": wTc, "bias": bc})

    res = run_bass_kernel_spmd(
        nc, in_maps, core_ids=list(range(n_cores)), trace=trace
    )
    out = np.empty((M, N), dtype=np.float32)
    for c in range(n_cores):
        out[:, c * N_shard : (c + 1) * N_shard] = res.results[c]["out"]
    return out.reshape(*lead_shape, N), res


_RUNNER_CACHE: dict = {}


def _cached_pjrt_run(M, K, N_shard, n_cores, in_maps):
    """Compile-once-per-shape PJRT executor (same machinery as
    run_bitnet_timed, which is HW-validated); repeat kernel() calls skip the
    multi-minute NEFF rebuild and only pay transfer + execution."""
    import jax
    import jax.numpy as jnp
    from jax.sharding import Mesh, NamedSharding, PartitionSpec
    from jax.experimental.shard_map import shard_map

    from concourse import bass2jax
    from concourse.bass2jax import _bass_exec_p, partition_id_tensor

    key = (M, K, N_shard, n_cores)
    if key not in _RUNNER_CACHE:
        bass2jax.install_neuronx_cc_hook()
        nc = build_bitnet_nc(M, K, N_shard, N_shard * n_cores * K, n_cores=n_cores)
        partition_name = (
            nc.partition_id_tensor.name if nc.partition_id_tensor else None
        )
        in_names, out_names, out_avals, zero_outs = [], [], [], []
        for alloc in nc.m.functions[0].allocations:
            if not isinstance(alloc, mybir.MemoryLocationSet):
                continue
            name = alloc.memorylocations[0].name
            if alloc.kind == "ExternalInput":
                if name != partition_name:
                    in_names.append(name)
            elif alloc.kind == "ExternalOutput":
                shape = tuple(alloc.tensor_shape)
                dtype = mybir.dt.np(alloc.dtype)
                out_names.append(name)
                out_avals.append(jax.core.ShapedArray(shape, dtype))
                zero_outs.append(np.zeros(shape, dtype))
        n_params = len(in_names)
        n_outs = len(out_avals)
        param_names = list(in_names)
        in_names = in_names + out_names
        if partition_name is not None:
            in_names.append(partition_name)
        donate = tuple(range(n_params, n_params + n_outs))

        def _body(*args):
            operands = list(args)
            if partition_name is not None:
                operands.append(partition_id_tensor())
            return tuple(
                _bass_exec_p.bind(
                    *operands,
                    out_avals=tuple(out_avals),
                    in_names=tuple(in_names),
                    out_names=tuple(out_names),
                    lowering_input_output_aliases=(),
                    sim_require_finite=True,
                    sim_require_nnan=True,
                    nc=nc,
                )
            )

        devices = jax.devices()[:n_cores]
        mesh = Mesh(np.asarray(devices), ("core",))
        sh = NamedSharding(mesh, PartitionSpec("core"))
        sharded = jax.jit(
            shard_map(
                _body,
                mesh=mesh,
                in_specs=(PartitionSpec("core"),) * (n_params + n_outs),
                out_specs=(PartitionSpec("core"),) * len(out_names),
                check_rep=False,
            ),
            donate_argnums=donate,
            keep_unused=True,
        )
        zfns = [
            jax.jit(
                lambda shp=(n_cores * z.shape[0], *z.shape[1:]),
                dt=z.dtype: jnp.zeros(shp, dt),
                out_shardings=sh,
            )
            for z in zero_outs
        ]
        _RUNNER_CACHE[key] = (sharded, param_names, out_names, out_avals, sh, zfns)

    sharded, param_names, out_names, out_avals, sh, zfns = _RUNNER_CACHE[key]
    import jax

    concat_in = [
        jax.device_put(
            np.concatenate(
                [np.asarray(in_maps[c][nm]) for c in range(n_cores)], 0
            ),
            sh,
        )
        for nm in param_names
    ]
    out_arrs = sharded(*concat_in, *[f() for f in zfns])
    oi = out_names.index("out")
    glob = np.asarray(out_arrs[oi]).reshape(n_cores, *out_avals[oi].shape)
    return [glob[c] for c in range(n_cores)]


def kernel(x: np.ndarray, weight: np.ndarray, bias: np.ndarray) -> np.ndarray:
    lead_shape = x.shape[:-1]
    K = x.shape[-1]
    N = weight.shape[0]
    M = int(np.prod(lead_shape))
    n_cores = 8
    N_shard = N // n_cores

    x2 = np.ascontiguousarray(x.reshape(M, K).astype(np.float32, copy=False))
    xT = np.ascontiguousarray(x2.T)
    w = weight.astype(np.float32, copy=False)
    in_maps = []
    for c in range(n_cores):
        in_maps.append(
            {
                "xT": xT,
                "wT": np.ascontiguousarray(w[c * N_shard : (c + 1) * N_shard, :].T),
                "bias": np.ascontiguousarray(
                    bias[c * N_shard : (c + 1) * N_shard]
                ).astype(np.float32, copy=False),
            }
        )
    shards = _cached_pjrt_run(M, K, N_shard, n_cores, in_maps)
    out = np.empty((M, N), dtype=np.float32)
    for c in range(n_cores):
        out[:, c * N_shard : (c + 1) * N_shard] = shards[c]
    return out.reshape(*lead_shape, N)


def run_bitnet_timed(
    x: np.ndarray,
    weight: np.ndarray,
    bias: np.ndarray,
    n_cores: int = 8,
    nsplits: int = 2,
    reps: int = 4,
    rounds: int = 6,
):
    """Like run_bitnet, but measures HW time via the reps-difference method:
    build the kernel once plain and once with the main loop unrolled `reps`
    times, time single dispatches of each (min over `rounds`), and divide the
    delta by reps-1.  This cancels the multi-ms, noisy axon dispatch floor.
    Returns (out, per_exec_seconds, diag)."""
    import time

    import jax
    import jax.numpy as jnp
    from jax.sharding import Mesh, NamedSharding, PartitionSpec
    from jax.experimental.shard_map import shard_map

    from concourse import bass2jax
    from concourse.bass2jax import _bass_exec_p, partition_id_tensor

    lead_shape = x.shape[:-1]
    K = x.shape[-1]
    N = weight.shape[0]
    M = int(np.prod(lead_shape))
    N_shard = N // n_cores

    x2 = np.ascontiguousarray(x.reshape(M, K).astype(np.float32, copy=False))
    xT = np.ascontiguousarray(x2.T)
    w = weight.astype(np.float32, copy=False)

    bass2jax.install_neuronx_cc_hook()

    in_maps = []
    for c in range(n_cores):
        wTc = np.ascontiguousarray(w[c * N_shard : (c + 1) * N_shard, :].T)
        bc = np.ascontiguousarray(bias[c * N_shard : (c + 1) * N_shard]).astype(
            np.float32, copy=False
        )
        in_maps.append({"xT": xT, "wT": wTc, "bias": bc})

    devices = jax.devices()[:n_cores]
    mesh = Mesh(np.asarray(devices), ("core",))
    sh = NamedSharding(mesh, PartitionSpec("core"))

    def make_runner(nc):
        partition_name = (
            nc.partition_id_tensor.name if nc.partition_id_tensor else None
        )
        in_names, out_names, out_avals, zero_outs = [], [], [], []
        for alloc in nc.m.functions[0].allocations:
            if not isinstance(alloc, mybir.MemoryLocationSet):
                continue
            name = alloc.memorylocations[0].name
            if alloc.kind == "ExternalInput":
                if name != partition_name:
                    in_names.append(name)
            elif alloc.kind == "ExternalOutput":
                shape = tuple(alloc.tensor_shape)
                dtype = mybir.dt.np(alloc.dtype)
                out_names.append(name)
                out_avals.append(jax.core.ShapedArray(shape, dtype))
                zero_outs.append(np.zeros(shape, dtype))
        n_params = len(in_names)
        n_outs = len(out_avals)
        in_names.extend(out_names)
        if partition_name is not None:
            in_names.append(partition_name)
        donate = tuple(range(n_params, n_params + n_outs))

        def _body(*args):
            operands = list(args)
            if partition_name is not None:
                operands.append(partition_id_tensor())
            return tuple(
                _bass_exec_p.bind(
                    *operands,
                    out_avals=tuple(out_avals),
                    in_names=tuple(in_names),
                    out_names=tuple(out_names),
                    lowering_input_output_aliases=(),
                    sim_require_finite=True,
                    sim_require_nnan=True,
                    nc=nc,
                )
            )

        sharded = jax.jit(
            shard_map(
                _body,
                mesh=mesh,
                in_specs=(PartitionSpec("core"),) * (n_params + n_outs),
                out_specs=(PartitionSpec("core"),) * len(out_names),
                check_rep=False,
            ),
            donate_argnums=donate,
            keep_unused=True,
        )
        concat_in = [
            jax.device_put(
                np.concatenate(
                    [np.asarray(in_maps[c][nm]) for c in range(n_cores)], 0
                ),
                sh,
            )
            for nm in in_names[:n_params]
        ]
        zfns = [
            jax.jit(
                lambda shp=(n_cores * z.shape[0], *z.shape[1:]), dt=z.dtype: jnp.zeros(
                    shp, dt
                ),
                out_shardings=sh,
            )
            for z in zero_outs
        ]

        def run_once():
            z = [f() for f in zfns]
            jax.block_until_ready(z)
            t0 = time.perf_counter()
            o = sharded(*concat_in, *z)
            jax.block_until_ready(o)
            return time.perf_counter() - t0, o

        return run_once, out_names

    nc1 = build_bitnet_nc(
        M, K, N_shard, N * K, n_cores=n_cores, nsplits=nsplits, reps=1
    )
    run1, out_names = make_runner(nc1)
    t_warm, out_arrs = run1()  # includes NEFF compile+load

    ncR = build_bitnet_nc(
        M, K, N_shard, N * K, n_cores=n_cores, nsplits=nsplits, reps=reps
    )
    runR, _ = make_runner(ncR)
    runR()  # warmup/compile

    t1s, tRs = [], []
    for _ in range(rounds):
        t1s.append(run1()[0])
        tRs.append(runR()[0])
    t1 = min(t1s)
    tR = min(tRs)
    per_exec = (tR - t1) / (reps - 1)
    diag = {"t1_min": t1, "tR_min": tR, "t1s": t1s, "tRs": tRs}

    oi = out_names.index("out")
    glob = np.asarray(out_arrs[oi]).reshape(n_cores, M, N_shard)
    out = np.empty((M, N), dtype=np.float32)
    for c in range(n_cores):
        out[:, c * N_shard : (c + 1) * N_shard] = glob[c]
    return out.reshape(*lead_shape, N), per_exec, diag



# revision 2
# speedup vs baseline: 3.6881x; 3.6881x over previous
"""BitNet linear (y = (x @ sign(W).T + b) * mean(|W|)) on 8 trn2 NeuronCores.

Sharding: column-parallel — W is sharded along out_features across the 8
cores, x is replicated, each core produces out[:, shard] and the host
concatenates.

Device algorithm (per core), mixed-precision along K:
  - The first K8 = k8_chunks*128 contraction dims use fp8e4 (E4M3) x with
    perf_mode=DoubleRow matmuls (2 k-chunks contracted per moving column
    -> ~2x bf16 MAC rate); the remaining dims use single-pass bf16 x.
    Weights are sign(W) in {-1,0,+1}: exact in fp8e4 AND bf16, so all
    quantization error comes from x.  Measured end-to-end L2 rel err vs
    the fp32 reference at K8=2048/4096: 1.85e-2 (gate: 2e-2).
  - The host only does dtype casts + layout (x -> fp8/bf16 block-images,
    W^T -> bf16); sign(W), mean|W| (AllReduce across cores), the matmul,
    bias add and alpha scale all run on device.
  - Per M-block (128 rows): 2 contiguous input DMAs (fp8 + bf16 image),
    8 DoubleRow + 16 bf16 matmuls into 4 PSUM banks, fused
    scale(alpha)+bias PSUM->SBUF op, 1 output DMA.  Next block's input
    DMAs are emitted before this block's PSUM drain so the PE never
    starves at block boundaries.
"""

import numpy as np
import ml_dtypes

import concourse.bass as bass
import concourse.mybir as mybir
import concourse.tile as tile
from concourse.bass import ds
from concourse.vector_clock import ScopedClock

# ---------------------------------------------------------------------------
# Compatibility patch: the pinned walrus (neuronxcc) in this container only
# supports ONE ge-wait per instruction and no eq-waits; the concourse Tile
# tail emits a Drain with multiple waits plus an eq-wait barrier butterfly
# ("Too many sync wait commands").  Replace the tail with one-wait-per-nop
# splitting and the NRT-expanded PSEUDO_SYNC_BARRIER (the pre-butterfly
# mechanism this walrus/NRT pair supports).
# ---------------------------------------------------------------------------


def _compat_drain_and_barrier(self, tick_clock, wait_clock):
    nc = self.nc
    coll = nc.sync.nop(nofuse=True)
    wait_clock.add_sem_waits(coll.ins, ScopedClock({None: tick_clock.global_clock}))
    si = coll.ins.sync_info
    if si is not None:
        waits = list(si.on_wait)
        if len(waits) > 1:
            coll.ins.sync_info = mybir.SyncInfo(
                on_wait=[waits[0]], on_update=list(si.on_update)
            )
            for w in waits[1:]:
                extra = nc.sync.nop(nofuse=True)
                extra.ins.sync_info = mybir.SyncInfo(on_wait=[w], on_update=[])
    for eng in nc.engines.values():
        eng.drain()
    nc._nrt_pseudo_barrier()
    popped = nc._tile_sem_poison_stack.pop()
    assert popped is self._sem_poison
    nc.clear_and_free_semaphores(list(self.sems.allocated().values()))
    nc._nrt_pseudo_barrier()


tile.TileContext._drain_and_barrier = _compat_drain_and_barrier

_legalize_ctr = [0]


def legalize_waits(nc):
    """Split instructions carrying more than the HW-supported number of sem
    waits (1; EventSemaphore: 2) into preceding one-wait NoOps on the same
    engine — semantically identical, encodable by the pinned walrus."""
    import bass_rust

    for f in nc.m.functions:
        for bb in f.blocks:
            il = bb.instructions
            i = 0
            while i < len(il):
                ins = il[i]
                si = ins.sync_info
                waits = list(si.on_wait) if si is not None else []
                limit = 2 if type(ins).__name__ == "InstEventSemaphore" else 1
                if len(waits) > limit:
                    keep = waits[-limit:]
                    spill = waits[:-limit]
                    for w in spill:
                        _legalize_ctr[0] += 1
                        nop = bass_rust.InstNoOp(
                            name=f"I-lw{_legalize_ctr[0]}", ins=[], outs=[]
                        )
                        nop.engine = ins.engine
                        nop.sync_info = mybir.SyncInfo(on_wait=[w], on_update=[])
                        il.insert(i, nop)
                        i += 1
                    ins.sync_info = mybir.SyncInfo(
                        on_wait=keep, on_update=list(si.on_update)
                    )
                i += 1


def elide_redundant_ldweights(nc):
    """Drop InstLdweights that reload the exact weights already sitting in
    the PE array.  bass lowers every InstMatmult to an Ldweights+Matmult
    pair; consecutive matmuls sharing one stationary tile reload it each
    time (~107ns of PE time apiece).  Two Ldweights with no other Ldweights
    between them and the same (tile name, offset, pattern) provably load
    identical content — tile names are unique per pool.tile() call and each
    tile is written before its first consumer only.  Elided instructions
    carrying semaphore waits/updates become NoOps to preserve sync."""
    import bass_rust

    n_elided = 0
    for f in nc.m.functions:
        for bb in f.blocks:
            il = bb.instructions
            last_key = None
            for i in range(len(il)):
                ins = il[i]
                nm = type(ins).__name__
                if nm != "InstLdweights":
                    continue
                a = ins.ins[0]
                bap = getattr(a, "bass_ap", None)
                if bap is None:
                    last_key = None
                    continue
                key = (
                    bap.tensor.name,
                    bap.offset,
                    str(bap.ap),
                    ins.perf_mode,
                    ins.is_transpose,
                    ins.tile_position,
                )
                if key == last_key:
                    si = ins.sync_info
                    has_sync = si is not None and (
                        list(si.on_wait) or list(si.on_update)
                    )
                    nop = bass_rust.InstNoOp(name=f"{ins.name}-eld", ins=[], outs=[])
                    nop.engine = ins.engine
                    if has_sync:
                        nop.sync_info = mybir.SyncInfo(
                            on_wait=list(si.on_wait), on_update=list(si.on_update)
                        )
                    il[i] = nop
                    n_elided += 1
                else:
                    last_key = key
    return n_elided


F32 = mybir.dt.float32
BF16 = mybir.dt.bfloat16
F8 = mybir.dt.float8e4

P = 128  # partitions
K8_CHUNKS_DEFAULT = 16  # fp8 region size in 128-chunks (of K/128 total)


def build_bitnet_nc(
    M: int,
    K: int,
    N_shard: int,
    n_total_weight: int,
    n_cores: int = 8,
    k8_chunks: int = K8_CHUNKS_DEFAULT,
    legalize: bool = True,
    reps: int = 1,
    skip_cc: bool = False,
    fuse_bias: bool = True,
):
    """Build the per-core Bass program.

    M: rows of x (B*S), K: in_features, N_shard: out_features per core.
    n_total_weight: total element count of the full W (for mean(|W|)).
    k8_chunks: leading 128-chunks of K computed in fp8-DoubleRow (even).
    """
    assert M % P == 0 and K % P == 0
    K_CHUNKS = K // P
    K8C = k8_chunks
    assert 0 <= K8C <= K_CHUNKS and K8C % 2 == 0
    KBC = K_CHUNKS - K8C
    NPAIR = K8C // 2
    N_TILE = min(512, N_shard)
    assert N_shard % N_TILE == 0
    NB = N_shard // N_TILE
    M_BLOCKS = M // P

    nc = bass.Bass(num_devices=n_cores)
    # host-prepared per-block SBUF images: [block, partition(k%128), chunk, m]
    x8_d = nc.declare_dram_parameter("x8", [M_BLOCKS, P, max(K8C, 1), P], F8,
                                     isOutput=False)
    xb_d = nc.declare_dram_parameter("xb", [M_BLOCKS, P, max(KBC, 1), P], BF16,
                                     isOutput=False)
    wT_d = nc.declare_dram_parameter("wT", [K, N_shard], BF16, isOutput=False)
    bias_d = nc.declare_dram_parameter("bias", [N_shard], F32, isOutput=False)
    out_d = nc.declare_dram_parameter("out", [M, N_shard], F32, isOutput=True)

    DR = mybir.MatmulPerfMode.DoubleRow

    with tile.TileContext(nc) as tc:
        wq_pool = tc.tile_pool(name="wq", bufs=1)
        wstage = tc.tile_pool(name="wstage", bufs=2)
        small = tc.tile_pool(name="small", bufs=1)
        x8_pool = tc.tile_pool(name="x8p", bufs=2)
        xb_pool = tc.tile_pool(name="xbp", bufs=2)
        out_pool = tc.tile_pool(name="outp", bufs=2)
        psum_pool = tc.tile_pool(name="psum", bufs=2, space="PSUM")
        apsum_pool = tc.tile_pool(name="apsum", bufs=1, space="PSUM")
        dram = tc.tile_pool(name="dram", bufs=1, space="DRAM")

        with (
            wq_pool as wq_p,
            wstage as wst_p,
            small as small_p,
            x8_pool as x8_p,
            xb_pool as xb_p,
            out_pool as out_p,
            psum_pool as ps_p,
            apsum_pool as aps_p,
            dram as dram_p,
        ):
            # ---------------- Phase A: sign(W) + |W| partial sums ----------
            # w^T arrives bf16; sign is exact in bf16 AND fp8 ({-1,0,+1}).
            wq8 = wq_p.tile([P, max(K8C, 1), N_shard], F8)
            wqb = wq_p.tile([P, max(KBC, 1), N_shard], BF16)
            acc = small_p.tile([P, K_CHUNKS], F32)
            abs_dump = small_p.tile([P, N_shard], F32)
            for k in range(K_CHUNKS):
                wst = wst_p.tile([P, N_shard], BF16, tag="wst")
                nc.sync.dma_start(wst[:], wT_d[k * P : (k + 1) * P, :])
                # per-chunk |W| sum on ScalarE while DVE does the sign clamp
                nc.scalar.activation(
                    abs_dump[:],
                    wst[:],
                    mybir.ActivationFunctionType.Abs,
                    accum_out=acc[:, k : k + 1],
                )
                # sign via clamp: s = max(min(w * 1e30, 1), -1), exact
                # {-1, 0, +1}; run in bf16 (no overflow: bf16 exponent range
                # matches fp32), then cast the fp8 region's chunk to fp8.
                if k < K8C:
                    sgn = wst_p.tile([P, N_shard], BF16, tag="sgn")
                    nc.vector.tensor_scalar(
                        sgn[:], wst[:], 1e30, 1.0,
                        mybir.AluOpType.mult, mybir.AluOpType.min,
                    )
                    nc.vector.tensor_scalar(
                        sgn[:], sgn[:], -1.0, None, mybir.AluOpType.max
                    )
                    nc.vector.tensor_copy(wq8[:, k, :], sgn[:])
                else:
                    kb = k - K8C
                    nc.vector.tensor_scalar(
                        wqb[:, kb, :], wst[:], 1e30, 1.0,
                        mybir.AluOpType.mult, mybir.AluOpType.min,
                    )
                    nc.vector.tensor_scalar(
                        wqb[:, kb, :], wqb[:, kb, :], -1.0, None,
                        mybir.AluOpType.max,
                    )

            # ---------------- Phase B: alpha = mean|W| over all cores ------
            asum = small_p.tile([P, 1], F32)
            nc.vector.reduce_sum(asum[:], acc[:], axis=mybir.AxisListType.X)
            ones_pp = small_p.tile([P, P], F32)
            nc.vector.memset(ones_pp[:], 1.0)
            aps = aps_p.tile([P, 1], F32)
            # ones^T @ asum: sum over partitions, broadcast to all partitions
            nc.tensor.matmul(aps[:], ones_pp[:], asum[:], start=True, stop=True)
            part_sum = small_p.tile([P, 1], F32)
            nc.vector.tensor_copy(part_sum[:], aps[:])

            cc_in = dram_p.tile([P, 1], F32)
            cc_out = dram_p.tile(
                [P, 1], F32, addr_space="Shared" if n_cores > 4 else "Local"
            )
            nc.sync.dma_start(cc_in[:], part_sum[:])
            if skip_cc:
                nc.sync.dma_start(cc_out[:], cc_in[:])
            else:
                nc.gpsimd.collective_compute(
                    "AllReduce",
                    mybir.AluOpType.add,
                    replica_groups=[list(range(n_cores))],
                    ins=[cc_in.opt()],
                    outs=[cc_out.opt()],
                )
            gsum = small_p.tile([P, 1], F32)
            nc.sync.dma_start(gsum[:], cc_out[:])
            alpha = small_p.tile([P, 1], F32)
            nc.vector.tensor_scalar_mul(alpha[:], gsum[:], 1.0 / float(n_total_weight))

            # bias: build a [128, N_shard] bf16 broadcast of bias*alpha ONCE
            # (ones-matmul broadcast); fused into the per-block PSUM->SBUF op
            bias_sb = small_p.tile([1, N_shard], F32)
            nc.sync.dma_start(bias_sb[:], bias_d[None, :])
            ones_row = small_p.tile([1, P], F32)
            nc.vector.memset(ones_row[:], 1.0)
            bias_bc = small_p.tile([P, N_shard], BF16)
            if fuse_bias:
                nc.vector.tensor_scalar_mul(bias_sb[:], bias_sb[:], alpha[:1, :])
                for n in range(NB):
                    bps = ps_p.tile([P, N_TILE], F32, tag="ps", name=f"bps{n}")
                    nc.tensor.matmul(
                        bps[:],
                        ones_row[:],
                        bias_sb[:, ds(n * N_TILE, N_TILE)],
                        start=True,
                        stop=True,
                    )
                    nc.vector.tensor_copy(bias_bc[:, ds(n * N_TILE, N_TILE)], bps[:])

            # ---------------- Phase C: main matmul -------------------------
            total_blocks = reps * M_BLOCKS

            def emit_in_dma(m, tag):
                tiles = []
                if K8C:
                    x8t = x8_p.tile([P, K8C, P], F8, tag="x8", name=f"x8{tag}")
                    nc.sync.dma_start(x8t[:], x8_d[m])
                    tiles.append(x8t)
                else:
                    tiles.append(None)
                if KBC:
                    xbt = xb_p.tile([P, KBC, P], BF16, tag="xb", name=f"xb{tag}")
                    nc.sync.dma_start(xbt[:], xb_d[m])
                    tiles.append(xbt)
                else:
                    tiles.append(None)
                return tiles

            pending = emit_in_dma(0, "b0")
            for bi in range(total_blocks):
                m = bi % M_BLOCKS
                x8t, xbt = pending

                psums = [
                    ps_p.tile([P, N_TILE], F32, tag="ps", name=f"ps{n}")
                    for n in range(NB)
                ]
                # fp8 DoubleRow pairs: contract chunks (2pp, 2pp+1) at once
                for pp in range(NPAIR):
                    lhsT = x8t[:, 2 * pp : 2 * pp + 2, :]
                    last = KBC == 0 and pp == NPAIR - 1
                    for n in range(NB):
                        nc.tensor.matmul(
                            psums[n][:],
                            lhsT,
                            wq8[:, 2 * pp : 2 * pp + 2, ds(n * N_TILE, N_TILE)],
                            start=pp == 0,
                            stop=last,
                            perf_mode=DR,
                        )
                # bf16 region
                for c in range(KBC):
                    lhsT = xbt[:, c, :]
                    last = c == KBC - 1
                    for n in range(NB):
                        nc.tensor.matmul(
                            psums[n][:],
                            lhsT,
                            wqb[:, c, ds(n * N_TILE, N_TILE)],
                            start=K8C == 0 and c == 0,
                            stop=last,
                        )

                # next block's input DMAs BEFORE this block's PSUM drain so
                # the DMA queue isn't stuck behind the output store
                if bi + 1 < total_blocks:
                    pending = emit_in_dma((bi + 1) % M_BLOCKS, f"b{bi + 1}")

                osb = out_p.tile([P, N_shard], F32, tag="osb")
                for n in range(NB):
                    if fuse_bias:
                        nc.vector.scalar_tensor_tensor(
                            osb[:, ds(n * N_TILE, N_TILE)],
                            psums[n][:],
                            alpha[:],
                            bias_bc[:, ds(n * N_TILE, N_TILE)],
                            mybir.AluOpType.mult,
                            mybir.AluOpType.add,
                        )
                    else:
                        nc.vector.tensor_scalar_mul(
                            osb[:, ds(n * N_TILE, N_TILE)], psums[n][:], alpha[:]
                        )
                nc.sync.dma_start(out_d[m * P : (m + 1) * P, :], osb[:])

    if legalize:
        legalize_waits(nc)  # required for walrus; CoreSim chokes on raw NoOps
    elide_redundant_ldweights(nc)
    return nc


def _host_prepare(x: np.ndarray, weight: np.ndarray, bias: np.ndarray,
                  n_cores: int, k8_chunks: int):
    """Host-side dtype casts + layout (no arithmetic beyond rounding):
    x -> per-block fp8/bf16 SBUF images (shared across cores), W^T -> bf16
    per-core shards."""
    lead_shape = x.shape[:-1]
    K = x.shape[-1]
    N = weight.shape[0]
    M = int(np.prod(lead_shape))
    assert weight.shape == (N, K) and bias.shape == (N,)
    assert N % n_cores == 0
    N_shard = N // n_cores
    K8 = k8_chunks * P
    KBC = K // P - k8_chunks
    M_BLOCKS = M // P

    x2 = np.ascontiguousarray(x.reshape(M, K).astype(np.float32, copy=False))
    # [mb, m, c, p] -> [mb, p, c, m]
    if k8_chunks:
        a = x2[:, :K8].astype(ml_dtypes.float8_e4m3)
        x8 = np.ascontiguousarray(
            a.reshape(M_BLOCKS, P, k8_chunks, P).transpose(0, 3, 2, 1)
        )
    else:
        x8 = np.zeros((M_BLOCKS, P, 1, P), dtype=ml_dtypes.float8_e4m3)
    if KBC:
        b = x2[:, K8:].astype(ml_dtypes.bfloat16)
        xb = np.ascontiguousarray(
            b.reshape(M_BLOCKS, P, KBC, P).transpose(0, 3, 2, 1)
        )
    else:
        xb = np.zeros((M_BLOCKS, P, 1, P), dtype=ml_dtypes.bfloat16)

    w = weight.astype(np.float32, copy=False)
    in_maps = []
    for c in range(n_cores):
        wTc = np.ascontiguousarray(
            w[c * N_shard : (c + 1) * N_shard, :].T.astype(ml_dtypes.bfloat16)
        )
        bc = np.ascontiguousarray(bias[c * N_shard : (c + 1) * N_shard]).astype(
            np.float32, copy=False
        )
        in_maps.append({"x8": x8, "xb": xb, "wT": wTc, "bias": bc})
    return in_maps, M, K, N, N_shard, lead_shape


def run_bitnet(
    x: np.ndarray,
    weight: np.ndarray,
    bias: np.ndarray,
    n_cores: int = 8,
    k8_chunks: int = K8_CHUNKS_DEFAULT,
    trace: bool = False,
):
    """Host driver: shard, run on n_cores, gather. x: [..., K], weight: [N, K]."""
    from concourse.bass_utils import run_bass_kernel_spmd

    in_maps, M, K, N, N_shard, lead_shape = _host_prepare(
        x, weight, bias, n_cores, k8_chunks
    )
    nc = build_bitnet_nc(M, K, N_shard, N * K, n_cores=n_cores,
                         k8_chunks=k8_chunks)
    res = run_bass_kernel_spmd(
        nc, in_maps, core_ids=list(range(n_cores)), trace=trace
    )
    out = np.empty((M, N), dtype=np.float32)
    for c in range(n_cores):
        out[:, c * N_shard : (c + 1) * N_shard] = res.results[c]["out"]
    return out.reshape(*lead_shape, N), res


_RUNNER_CACHE: dict = {}


def _make_runner(nc, n_cores, in_map_names=None):
    """Compile a sharded PJRT executor for the given Bass program."""
    import jax
    import jax.numpy as jnp
    from jax.sharding import Mesh, NamedSharding, PartitionSpec
    from jax.experimental.shard_map import shard_map

    from concourse import bass2jax
    from concourse.bass2jax import _bass_exec_p, partition_id_tensor

    bass2jax.install_neuronx_cc_hook()
    partition_name = nc.partition_id_tensor.name if nc.partition_id_tensor else None
    in_names, out_names, out_avals, zero_outs = [], [], [], []
    for alloc in nc.m.functions[0].allocations:
        if not isinstance(alloc, mybir.MemoryLocationSet):
            continue
        name = alloc.memorylocations[0].name
        if alloc.kind == "ExternalInput":
            if name != partition_name:
                in_names.append(name)
        elif alloc.kind == "ExternalOutput":
            shape = tuple(alloc.tensor_shape)
            dtype = mybir.dt.np(alloc.dtype)
            out_names.append(name)
            out_avals.append(jax.core.ShapedArray(shape, dtype))
            zero_outs.append(np.zeros(shape, dtype))
    n_params = len(in_names)
    n_outs = len(out_avals)
    param_names = list(in_names)
    in_names = in_names + out_names
    if partition_name is not None:
        in_names.append(partition_name)
    donate = tuple(range(n_params, n_params + n_outs))

    def _body(*args):
        operands = list(args)
        if partition_name is not None:
            operands.append(partition_id_tensor())
        return tuple(
            _bass_exec_p.bind(
                *operands,
                out_avals=tuple(out_avals),
                in_names=tuple(in_names),
                out_names=tuple(out_names),
                lowering_input_output_aliases=(),
                sim_require_finite=True,
                sim_require_nnan=True,
                nc=nc,
            )
        )

    devices = jax.devices()[:n_cores]
    mesh = Mesh(np.asarray(devices), ("core",))
    sh = NamedSharding(mesh, PartitionSpec("core"))
    sharded = jax.jit(
        shard_map(
            _body,
            mesh=mesh,
            in_specs=(PartitionSpec("core"),) * (n_params + n_outs),
            out_specs=(PartitionSpec("core"),) * len(out_names),
            check_rep=False,
        ),
        donate_argnums=donate,
        keep_unused=True,
    )
    zfns = [
        jax.jit(
            lambda shp=(n_cores * z.shape[0], *z.shape[1:]),
            dt=z.dtype: jnp.zeros(shp, dt),
            out_shardings=sh,
        )
        for z in zero_outs
    ]
    return sharded, param_names, out_names, out_avals, sh, zfns


def _cached_pjrt_run(M, K, N_shard, n_cores, k8_chunks, in_maps):
    """Compile-once-per-shape PJRT executor; repeat kernel() calls skip the
    multi-minute NEFF rebuild and only pay transfer + execution."""
    import jax

    key = (M, K, N_shard, n_cores, k8_chunks)
    if key not in _RUNNER_CACHE:
        nc = build_bitnet_nc(M, K, N_shard, N_shard * n_cores * K,
                             n_cores=n_cores, k8_chunks=k8_chunks)
        _RUNNER_CACHE[key] = _make_runner(nc, n_cores)

    sharded, param_names, out_names, out_avals, sh, zfns = _RUNNER_CACHE[key]

    concat_in = [
        jax.device_put(
            np.concatenate(
                [np.asarray(in_maps[c][nm]) for c in range(n_cores)], 0
            ),
            sh,
        )
        for nm in param_names
    ]
    out_arrs = sharded(*concat_in, *[f() for f in zfns])
    oi = out_names.index("out")
    glob = np.asarray(out_arrs[oi]).reshape(n_cores, *out_avals[oi].shape)
    return [glob[c] for c in range(n_cores)]


def kernel(x: np.ndarray, weight: np.ndarray, bias: np.ndarray) -> np.ndarray:
    n_cores = 8
    k8_chunks = K8_CHUNKS_DEFAULT
    in_maps, M, K, N, N_shard, lead_shape = _host_prepare(
        x, weight, bias, n_cores, k8_chunks
    )
    shards = _cached_pjrt_run(M, K, N_shard, n_cores, k8_chunks, in_maps)
    out = np.empty((M, N), dtype=np.float32)
    for c in range(n_cores):
        out[:, c * N_shard : (c + 1) * N_shard] = shards[c]
    return out.reshape(*lead_shape, N)


def run_bitnet_timed(
    x: np.ndarray,
    weight: np.ndarray,
    bias: np.ndarray,
    n_cores: int = 8,
    nsplits: int = 2,  # kept for test.py signature compat; unused
    reps: int = 4,
    rounds: int = 6,
    k8_chunks: int = K8_CHUNKS_DEFAULT,
):
    """Like run_bitnet, but measures HW time via the reps-difference method:
    build the kernel once plain and once with the main loop unrolled `reps`
    times, time single dispatches of each (min over `rounds`), and divide the
    delta by reps-1.  This cancels the multi-ms, noisy axon dispatch floor.
    Returns (out, per_exec_seconds, diag)."""
    import time

    import jax

    in_maps, M, K, N, N_shard, lead_shape = _host_prepare(
        x, weight, bias, n_cores, k8_chunks
    )

    def runner_for(reps_):
        nc = build_bitnet_nc(M, K, N_shard, N * K, n_cores=n_cores,
                             k8_chunks=k8_chunks, reps=reps_)
        sharded, param_names, out_names, out_avals, sh, zfns = _make_runner(
            nc, n_cores
        )
        concat_in = [
            jax.device_put(
                np.concatenate(
                    [np.asarray(in_maps[c][nm]) for c in range(n_cores)], 0
                ),
                sh,
            )
            for nm in param_names
        ]

        def run_once():
            z = [f() for f in zfns]
            jax.block_until_ready(z)
            t0 = time.perf_counter()
            o = sharded(*concat_in, *z)
            jax.block_until_ready(o)
            return time.perf_counter() - t0, o

        return run_once, out_names, out_avals

    run1, out_names, out_avals = runner_for(1)
    t_warm, out_arrs = run1()  # includes NEFF compile+load

    runR, _, _ = runner_for(reps)
    runR()  # warmup/compile

    t1s, tRs = [], []
    for _ in range(rounds):
        t1s.append(run1()[0])
        tRs.append(runR()[0])
    t1 = min(t1s)
    tR = min(tRs)
    per_exec = (tR - t1) / (reps - 1)
    diag = {"t1_min": t1, "tR_min": tR, "t1s": t1s, "tRs": tRs}

    oi = out_names.index("out")
    glob = np.asarray(out_arrs[oi]).reshape(n_cores, M, N_shard)
    out = np.empty((M, N), dtype=np.float32)
    for c in range(n_cores):
        out[:, c * N_shard : (c + 1) * N_shard] = glob[c]
    return out.reshape(*lead_shape, N), per_exec, diag
